# revision 2
# baseline (speedup 1.0000x reference)
"""Fused 3-stage axial attention kernel for 8 NeuronCores.

Sharding: stage 0 (attention along d) shards (b, h/4); one on-device
AllToAll per 4-core group swaps h-quarters for d-quarters; stages 1
(along h) and 2 (along w) then run locally, handing off through SBUF.

Per-core geometry per stage: 12 "macro-tiles" of a 48x48 token grid
(2304 tokens); sequences of length 48 along one grid axis.

Key layout rules (hardware: engine-op partition bases must be 0 or 64):
  - x_sb feature-major [c-half=128, tok, seq] in DMA-natural order.
  - attention-side tiles compacted seq-major; xh_feat padded to 64
    cols/seq so 2-seq V-GEMM outputs land at partition bases {0, 64}.
  - scores computed transposed (s^T = k^T q) so exp directly yields A^T;
    the AV moving operand carries an appended ones column which produces
    the softmax denominator in the same matmul.
"""
import os
import sys

sys.path.insert(0, "/opt/trn_rl_repo")

KN_DEBUG = os.environ.get("KN_DEBUG", "")
KN_SKIP = set(os.environ.get("KN_SKIP", "").split(","))

import numpy as np
import ml_dtypes

import concourse.bass as bass
import concourse.bacc as bacc
import concourse.tile as tile
from concourse import mybir
from concourse.masks import make_identity
from concourse.bass_utils import run_bass_kernel_spmd

BF16 = ml_dtypes.bfloat16
f32 = mybir.dt.float32
bf16 = mybir.dt.bfloat16
AF = mybir.ActivationFunctionType
ALU = mybir.AluOpType

C = 256
S = 48
NH = 8
HD = 32
NSEQ = 48
NTOK = NSEQ * S      # 2304
NMACRO = 12
SCALE = 1.0 / np.sqrt(HD)
EPS = 1e-5

NG = NSEQ // 2       # 24 two-seq groups / seq pairs per macro
NSC = 8              # gemm sub-chunks per macro
SCW = NTOK // NSC    # 288 cols per sub-chunk (6 seqs)

_NC_CACHE = {}


def build_program():
    key = ("nc", KN_DEBUG, tuple(sorted(KN_SKIP)))
    if key in _NC_CACHE:
        return _NC_CACHE[key]
    nc = bacc.Bacc(num_devices=8)

    xin = nc.declare_dram_parameter("xin", [2, C, 48, 6, 48], bf16, isOutput=False)
    wq = nc.declare_dram_parameter("wq", [3, 2, 128, 384], bf16, isOutput=False)
    wk = nc.declare_dram_parameter("wk", [3, 2, 128, 384], bf16, isOutput=False)
    wv = nc.declare_dram_parameter("wv", [3, 2, 128, 256], bf16, isOutput=False)
    wo = nc.declare_dram_parameter("wo", [3, 2, 128, 256], bf16, isOutput=False)
    bq = nc.declare_dram_parameter("bq", [3, 128, 4], f32, isOutput=False)
    bk = nc.declare_dram_parameter("bk", [3, 128, 4], f32, isOutput=False)
    bo = nc.declare_dram_parameter("bo", [3, 128, 2], f32, isOutput=False)
    yout = nc.declare_dram_parameter("yout", [2, C, 6, 48, 48], bf16, isOutput=True)

    with tile.TileContext(nc) as tc:
        with (
            tc.tile_pool(name="consts", bufs=1) as consts,
            tc.tile_pool(name="xsb", bufs=1) as xsb_p,
            tc.tile_pool(name="work", bufs=1) as work,
            tc.tile_pool(name="att", bufs=1) as attp,
            tc.tile_pool(name="dram", bufs=1, space="DRAM") as dram,
            tc.tile_pool(name="pg", bufs=1, space="PSUM") as pg_p,
            tc.tile_pool(name="pa", bufs=1, space="PSUM") as pa_p,
            tc.tile_pool(name="pt", bufs=1, space="PSUM") as pt_p,
        ):
            # ---------------- constants + weights ----------------
            ident = consts.tile([128, 128], bf16, tag="ident", name="ident")
            make_identity(nc, ident)
            eps_t = consts.tile([128, 1], f32, tag="eps", name="eps")
            nc.vector.memset(eps_t, EPS)
            zb = consts.tile([128, 1], f32, tag="zb", name="zb")
            nc.vector.memset(zb, 0.0)

            def ldw(name, src, w=256):
                ts = []
                for s in range(3):
                    t = consts.tile([128, 2, w], bf16, tag=f"{name}{s}",
                                    name=f"{name}{s}")
                    for ch in range(2):
                        nc.sync.dma_start(out=t[:, ch, :], in_=src[s, ch])
                    ts.append(t)
                return ts

            def ldb(name, src, w=2):
                ts = []
                for s in range(3):
                    t = consts.tile([128, w], f32, tag=f"{name}{s}",
                                    name=f"{name}{s}")
                    nc.sync.dma_start(out=t, in_=src[s])
                    ts.append(t)
                return ts

            wq_s, wk_s = ldw("wq", wq, 384), ldw("wk", wk, 384)
            wv_s, wo_s = ldw("wv", wv), ldw("wo", wo)
            bq_s, bk_s = ldb("bq", bq, 4), ldb("bk", bk, 4)
            bo_s = ldb("bo", bo)

            # ---------------- SBUF working tiles ----------------
            def mk(pool, n, shape, dt_, nm):
                return [pool.tile(shape, dt_, tag=f"{nm}_{i}", name=f"{nm}_{i}")
                        for i in range(n)]

            x_sb = [mk(xsb_p, 2, [128, NTOK], bf16, f"xsb{ch}") for ch in range(2)]
            y2_sb = [xsb_p.tile([128, NTOK], f32, tag=f"y2_{ch}", name=f"y2_{ch}")
                     for ch in range(2)]
            xb = [work.tile([128, NTOK], bf16, tag=f"xb{ch}", name=f"xb{ch}")
                  for ch in range(2)]
            xh_tok = mk(work, 2, [96, 256], bf16, "xhtok")
            xh_feat = [mk(work, 2, [128, NSEQ * 64], bf16, f"xhf{ch}")
                       for ch in range(2)]
            # 4 tiles, head-pair per tile: rows 0:32 = even head,
            # 32:64 = zeros (zero weight cols), 64:96 = odd head
            q_feat = [work.tile([96, NTOK], bf16, tag=f"qf{t}", name=f"qf{t}")
                      for t in range(4)]
            k_feat = [work.tile([96, NTOK], bf16, tag=f"kf{t}", name=f"kf{t}")
                      for t in range(4)]
            o_feat = [work.tile([128, NTOK], bf16, tag=f"of{ch}", name=f"of{ch}")
                      for ch in range(2)]
            y_sb = [mk(work, 2, [128, SCW], bf16, f"ysb{ch}") for ch in range(2)]

            v_sb = mk(attp, 3, [112, NH * 36], bf16, "vsb")
            scr = mk(attp, 2, [112, NH * 96], bf16, "scr")
            o_sb = mk(attp, 2, [96, 256], bf16, "osb")
            st_t = mk(attp, 2, [96, 12], f32, "st")
            mv_t = mk(attp, 2, [96, 2], f32, "mv")
            std_t = mk(attp, 2, [96, 1], f32, "std")
            rstd_t = mk(attp, 2, [96, 1], f32, "rstd")
            rec_t = mk(attp, 2, [96, 8], f32, "rec")

            for t in scr:
                nc.gpsimd.memset(t, 0.0)
            for ch in range(2):
                for t in xh_feat[ch]:
                    nc.gpsimd.memset(t, 0.0)
            for t in v_sb:
                nc.gpsimd.memset(t, 0.0)
                oc = t.rearrange("p (h x) -> p h x", h=NH)[:, :, 32:33]
                nc.gpsimd.memset(oc[0:48], 1.0)
                nc.gpsimd.memset(oc[64:112], 1.0)
            AVW = 36

            y1b = dram.tile([8, 2, C, 6, 6, 48], bf16, name="y1b")
            a2a = dram.tile([8, 2, C, 6, 6, 48], bf16, name="a2a")

            # PSUM: 2(pg) + 2(pa) + 2(psc) + 2(pt) = 8 banks
            pg = mk(pg_p, 2, [128, 512], f32, "pg")      # q/k/proj gemms
            pa = mk(pa_p, 2, [128, 384], f32, "pa")      # v / av
            psc = mk(pa_p, 2, [112, 192], f32, "psc")    # scores, one bank per row-group
            ptc = mk(pt_p, 2, [128, 448], bf16, "ptc")   # [0:96,0:256]=A, [:,256:448]=B
            ptA = [t[0:96, 0:256] for t in ptc]
            ptB = [t[:, 256:448] for t in ptc]

            cnt = {"pg": 0, "pa": 0, "ptA": 0, "ptB": 0}

            def nx(nm, lst):
                t = lst[cnt[nm] % len(lst)]
                cnt[nm] += 1
                return t

            # ---------------- one stage macro ----------------
            def macro(stage, m, load, store):
                wq_t, wk_t, wv_t, wo_t = (wq_s[stage], wk_s[stage],
                                          wv_s[stage], wo_s[stage])
                seq_major = (stage == 2)
                xs = [load(ch) for ch in range(2)]
                xhf = [xh_feat[ch][m % 2] for ch in range(2)]

                # gather into compact seq-major bf16 (stage 2: also f32->bf16)
                for ch in range(2):
                    xv = xs[ch].rearrange("p (a b) -> p a b", a=S)
                    nc.vector.tensor_copy(
                        out=xb[ch].rearrange("p (a b) -> p a b", a=S),
                        in_=xv if seq_major else xv.transpose([0, 2, 1]))

                def ln_src(ch, g):
                    return xb[ch][:, g * 96:(g + 1) * 96]

                # LN per 2-seq group
                for g in range(NG):
                    tp = nx("ptA", ptA)
                    st, mv = st_t[g % 2], mv_t[g % 2]
                    std, rstd, xht = std_t[g % 2], rstd_t[g % 2], xh_tok[g % 2]
                    for ch in range(2):
                        nc.tensor.matmul(
                            tp[:, ch * 128:(ch + 1) * 128],
                            ln_src(ch, g), ident,
                            is_transpose=True)
                        nc.vector.bn_stats(
                            out=st[:, ch * 6:(ch + 1) * 6],
                            in_=tp[:, ch * 128:(ch + 1) * 128])
                    nc.vector.bn_aggr(
                        out=mv, in_=st.rearrange("p (g s) -> p g s", g=2))
                    nc.scalar.activation(out=std, in_=mv[:, 1:2], func=AF.Sqrt,
                                         bias=eps_t[0:96], scale=1.0)
                    nc.vector.reciprocal(out=rstd, in_=std)
                    for ch in range(2):
                        nc.vector.tensor_scalar(
                            out=xht[:, ch * 128:(ch + 1) * 128],
                            in0=tp[:, ch * 128:(ch + 1) * 128],
                            scalar1=mv[:, 0:1], scalar2=rstd,
                            op0=ALU.subtract, op1=ALU.mult)
                    tf = nx("ptB", ptB)
                    for ch in range(2):
                        nc.tensor.matmul(
                            tf[:, ch * 96:(ch + 1) * 96],
                            xht[:, ch * 128:(ch + 1) * 128],
                            ident[0:96, 0:96], is_transpose=True)
                        dst = xhf[ch].rearrange(
                            "p (sq x) -> p sq x", x=64)[:, 2 * g:2 * g + 2, 0:48]
                        nc.scalar.copy(
                            out=dst,
                            in_=tf[:, ch * 96:(ch + 1) * 96].rearrange(
                                "p (sq k) -> p sq k", sq=2))

                if KN_DEBUG == "ln":
                    return
                # q/k gemms over sub-chunks
                for sc in range(NSC):
                    mov = [xhf[ch].rearrange(
                        "p (sq x) -> p sq x", x=64)[:, 6 * sc:6 * sc + 6, 0:48]
                        for ch in range(2)]
                    for wt, bt, dstf in ((wq_t, bq_s[stage], q_feat),
                                         (wk_t, bk_s[stage], k_feat)):
                        for t in range(4):
                            f0 = 96 * t
                            ps = nx("pg", pg)
                            nc.tensor.matmul(
                                ps[0:96, 0:SCW], wt[:, 0, f0:f0 + 96],
                                mov[0], start=True, stop=False)
                            nc.tensor.matmul(
                                ps[0:96, 0:SCW], wt[:, 1, f0:f0 + 96],
                                mov[1], start=False, stop=True)
                            nc.vector.tensor_scalar(
                                out=dstf[t][:, sc * SCW:(sc + 1) * SCW],
                                in0=ps[0:96, 0:SCW], scalar1=bt[0:96, t:t + 1],
                                scalar2=None, op0=ALU.add)

                if KN_DEBUG == "qk":
                    return
                # attention per seq pair
                for p in range(NG):
                    vs = v_sb[p % 3]
                    vps = nx("pa", pa)
                    nc.tensor.matmul(vps[:, 0:256],
                                     xhf[0][:, p * 128:(p + 1) * 128],
                                     wv_t[:, 0, :], start=True, stop=False)
                    nc.tensor.matmul(vps[:, 0:256],
                                     xhf[1][:, p * 128:(p + 1) * 128],
                                     wv_t[:, 1, :], start=False, stop=True)
                    for base in ((0, 64) if "vev" not in KN_SKIP else ()):
                        nc.vector.tensor_copy(
                            out=vs[base:base + 48].rearrange(
                                "p (h x) -> p h x", h=NH)[:, :, 0:32],
                            in_=vps[base:base + 48, 0:256].rearrange(
                                "p (h x) -> p h x", h=NH))
                    if KN_DEBUG == "v":
                        continue
                    for h in range(NH if "sco" not in KN_SKIP else 0):
                        t, r, g = h // 2, 64 * (h % 2), h % 2
                        for si, ob in ((2 * p, 0), (2 * p + 1, 64)):
                            nc.tensor.matmul(
                                psc[g][ob:ob + 48, t * 48:(t + 1) * 48],
                                k_feat[t][r:r + 32, si * S:(si + 1) * S],
                                q_feat[t][r:r + 32, si * S:(si + 1) * S],
                                start=True, stop=True)
                    sc_t = scr[p % 2]
                    if "exp" not in KN_SKIP:
                        scv = sc_t.rearrange("p (i x) -> p i x", i=4)
                        for g in range(2):
                            nc.scalar.activation(
                                out=scv[0:48][:, :, 96 * g:96 * g + 48],
                                in_=psc[g][0:48].rearrange(
                                    "p (t x) -> p t x", t=4),
                                func=AF.Exp, bias=zb[0:48], scale=1.0)
                            nc.scalar.activation(
                                out=scv[64:112][:, :, 96 * g + 48:96 * g + 96],
                                in_=psc[g][64:112].rearrange(
                                    "p (t x) -> p t x", t=4),
                                func=AF.Exp, bias=zb[64:112], scale=1.0)
                    if KN_DEBUG == "sc":
                        continue
                    ops = nx("pa", pa)
                    for h in range(NH if "av" not in KN_SKIP else 0):
                        nc.tensor.matmul(
                            ops[0:96, h * AVW:h * AVW + 33],
                            sc_t[:, h * 96:(h + 1) * 96],
                            vs[:, h * AVW:h * AVW + 33],
                            start=True, stop=True)
                    rec = rec_t[p % 2]
                    if "rec" in KN_SKIP:
                        nc.vector.memset(rec, 1.0)
                    else:
                        nc.vector.reciprocal(
                        out=rec, in_=ops[0:96, 0:NH * AVW].rearrange(
                            "p (h x) -> p h x", h=NH)[:, :, 32:33])
                    osb = o_sb[p % 2]
                    for h in range(NH if "oev" not in KN_SKIP else 0):
                        nc.vector.tensor_scalar(
                            out=osb[:, h * 32:(h + 1) * 32],
                            in0=ops[0:96, h * AVW:h * AVW + 32],
                            scalar1=rec[:, h:h + 1], scalar2=None, op0=ALU.mult)
                    tf = nx("ptB", ptB)
                    for ch in range(2 if "otr" not in KN_SKIP else 0):
                        nc.tensor.matmul(
                            tf[:, ch * 96:(ch + 1) * 96],
                            osb[:, ch * 128:(ch + 1) * 128],
                            ident[0:96, 0:96], is_transpose=True)
                        if seq_major:
                            dst = o_feat[ch][:, p * 96:(p + 1) * 96].rearrange(
                                "p (sq k) -> p sq k", sq=2)
                        else:
                            dst = o_feat[ch].rearrange(
                                "p (a b) -> p a b", a=S)[:, :, 2 * p:2 * p + 2] \
                                .transpose([0, 2, 1])
                        nc.scalar.copy(
                            out=dst,
                            in_=tf[:, ch * 96:(ch + 1) * 96].rearrange(
                                "p (sq k) -> p sq k", sq=2))

                if KN_DEBUG == "att":
                    return
                # output projection + residual per sub-chunk
                for sc in range(NSC):
                    for oh in range(2):
                        ps = nx("pg", pg)
                        nc.tensor.matmul(ps[:, 0:SCW],
                                         wo_t[:, 0, oh * 128:(oh + 1) * 128],
                                         o_feat[0][:, sc * SCW:(sc + 1) * SCW],
                                         start=True, stop=False)
                        nc.tensor.matmul(ps[:, 0:SCW],
                                         wo_t[:, 1, oh * 128:(oh + 1) * 128],
                                         o_feat[1][:, sc * SCW:(sc + 1) * SCW],
                                         start=False, stop=True)
                        store(oh, sc, ps[:, 0:SCW], bo_s[stage][:, oh:oh + 1],
                              xs[oh])

            # ---------------- stage drivers ----------------
            for m in range(NMACRO):
                bb, h0 = m // 6, m % 6

                def load0(ch, bb=bb, h0=h0, m=m):
                    t = x_sb[ch][m % 2]
                    nc.sync.dma_start(
                        out=t.rearrange("p (a b) -> p a b", a=S),
                        in_=xin[bb, ch * 128:(ch + 1) * 128, :, h0, :])
                    return t

                def store0(oh, sc, ps, bo_ap, xres, bb=bb, h0=h0, sc_i=None):
                    ysb = y_sb[oh][sc % 2]
                    nc.vector.scalar_tensor_tensor(
                        out=ysb, in0=ps, scalar=bo_ap,
                        in1=xres[:, sc * SCW:(sc + 1) * SCW],
                        op0=ALU.add, op1=ALU.add)
                    nc.sync.dma_start(
                        out=y1b[sc, bb, oh * 128:(oh + 1) * 128, :, h0, :],
                        in_=ysb.rearrange("p (a b) -> p a b", b=48))
                macro(0, m, load0, store0)

            s0only = KN_DEBUG in ("ln", "qk", "v", "sc", "att", "s0")
            if not s0only:
                if KN_DEBUG != "nocc":
                    nc.gpsimd.collective_compute(
                        "AllToAll", ALU.bypass,
                        replica_groups=[[0, 1, 2, 3, 4, 5, 6, 7]],
                        ins=[y1b.opt()], outs=[a2a.opt()],
                    )
                else:
                    nc.sync.dma_start(out=a2a[:], in_=y1b[:])

            for m in range(NMACRO if not s0only else 0):
                bb, d0 = m // 6, m % 6

                def load1(ch, bb=bb, d0=d0, m=m):
                    t = x_sb[ch][m % 2]
                    nc.sync.dma_start(
                        out=t.rearrange("p (a b c) -> p a b c", a=8, b=6),
                        in_=a2a[:, bb, ch * 128:(ch + 1) * 128, d0, :, :]
                        .transpose([1, 0, 2, 3]))
                    return t

                def store1(oh, sc, ps, bo_ap, xres):
                    nc.vector.scalar_tensor_tensor(
                        out=y2_sb[oh][:, sc * SCW:(sc + 1) * SCW], in0=ps,
                        scalar=bo_ap, in1=xres[:, sc * SCW:(sc + 1) * SCW],
                        op0=ALU.add, op1=ALU.add)
                macro(1, m, load1, store1)

                def load2(ch):
                    return y2_sb[ch]

                def store2(oh, sc, ps, bo_ap, xres, bb=bb, d0=d0):
                    ysb = y_sb[oh][sc % 2]
                    nc.vector.scalar_tensor_tensor(
                        out=ysb, in0=ps, scalar=bo_ap,
                        in1=xres[:, sc * SCW:(sc + 1) * SCW],
                        op0=ALU.add, op1=ALU.add)
                    nc.sync.dma_start(
                        out=yout[bb, oh * 128:(oh + 1) * 128, d0,
                                 sc * 6:sc * 6 + 6, :],
                        in_=ysb.rearrange("p (a b) -> p a b", b=48))
                macro(2, m, load2, store2)

    nc.finalize()
    _NC_CACHE[key] = nc
    return nc


def _prep_weights(inputs):
    gamma = float(np.asarray(inputs["gamma"]).reshape(-1)[0])
    out = {k: [] for k in ("wq", "wk", "wv", "wo", "bq", "bk", "bo")}
    for pre in ("d", "h", "w"):
        nw = np.asarray(inputs[f"{pre}n_w"], np.float32)
        nb = np.asarray(inputs[f"{pre}n_b"], np.float32)
        qw = np.asarray(inputs[f"{pre}q_w"], np.float32)
        qb = np.asarray(inputs[f"{pre}q_b"], np.float32)
        ow = np.asarray(inputs[f"{pre}o_w"], np.float32)
        ob = np.asarray(inputs[f"{pre}o_b"], np.float32)
        W = qw * nw[None, :]
        b = qb + qw @ nb
        Wq, bqv = W[0:256] * SCALE, b[0:256] * SCALE
        Wk, bkv = W[256:512], b[256:512]
        Wv, bvv = W[512:768], b[512:768]
        def pack4w(Wt):
            # [256c, 256f] -> [256c, 384]: tile t = [head 2t | zeros | head 2t+1]
            a = np.zeros((256, 384), np.float32)
            for t in range(4):
                a[:, t * 96:t * 96 + 32] = Wt[:, (2 * t) * 32:(2 * t + 1) * 32]
                a[:, t * 96 + 64:t * 96 + 96] = Wt[:, (2 * t + 1) * 32:(2 * t + 2) * 32]
            return a.reshape(2, 128, 384)
        out["wq"].append(pack4w(np.ascontiguousarray(Wq.T)))
        out["wk"].append(pack4w(np.ascontiguousarray(Wk.T)))
        out["wv"].append(np.ascontiguousarray(Wv.T).reshape(2, 128, 256))
        out["wo"].append(np.ascontiguousarray(gamma * ow.T).reshape(2, 128, 256))
        def pack4b(bv):
            a = np.zeros((128, 4), np.float32)
            for t in range(4):
                a[0:32, t] = bv[(2 * t) * 32:(2 * t + 1) * 32]
                a[64:96, t] = bv[(2 * t + 1) * 32:(2 * t + 2) * 32]
            return a
        out["bq"].append(pack4b(bqv))
        out["bk"].append(pack4b(bkv))
        out["bo"].append(
            np.ascontiguousarray((gamma * (ob + bvv @ ow.T)).reshape(2, 128).T))
    wd = {}
    for k in ("wq", "wk", "wv", "wo"):
        wd[k] = np.ascontiguousarray(np.stack(out[k]).astype(BF16))
    for k in ("bq", "bk", "bo"):
        wd[k] = np.ascontiguousarray(np.stack(out[k]).astype(np.float32))
    return wd


def kernel(**inputs):
    x = np.asarray(inputs["x"], np.float32)
    nc = build_program()
    wd = _prep_weights(inputs)

    xb16 = x.astype(BF16)
    in_maps = []
    for core in range(8):
        m = dict(wd)
        m["xin"] = np.ascontiguousarray(xb16[:, :, :, core * 6:(core + 1) * 6, :])
        in_maps.append(m)

    res = run_bass_kernel_spmd(nc, in_maps, list(range(8)))

    out = np.empty_like(x)
    for core in range(8):
        out[:, :, core * 6:(core + 1) * 6, :, :] = \
            res.results[core]["yout"].astype(np.float32)
    return out
